# revision 7
# baseline (speedup 1.0000x reference)
"""Multi-head attention (B=4, N=2048, C=768, H=12, D=64) on 8 Trainium2 cores.

Sharding: core i handles batch i//2 and head-group i%2 (6 heads each).
Each core computes its QKV slice, attention for its 6 heads, and a partial
output projection. The host sums the two per-batch partials and adds the
bias corrections (V-bias commutes through softmax since rows sum to 1, so
``qkv_b[v] @ proj_w.T + proj_b`` is added once on the host).

Device layouts (all matmul operands bf16, fp32 PSUM accumulation):
  xT      [C, N]        x[b].T
  qkvwT   [C, 1152]     weight columns ordered [q(6*64) | k(6*64) | v(6*64)]
  Q^T/K^T [768, N]      computed transposed so attention needs no on-chip transpose
  V'      [N, 6, 65]    V in [seq, feat] order with a ones column per head:
                        the attn@V matmul then yields softmax denominators in row 64.
Softmax skips max-subtraction (logits are within +-2 for this problem).
"""

import numpy as np
import ml_dtypes

B, N, C, H, D = 4, 2048, 768, 12, 64
HPG = 6            # heads per group (per core)
FG = HPG * D       # 384 features per head-group
CC = C // 128      # 6 contraction chunks
NT = N // 128      # 16 seq chunks of 128
NQ = N // 512      # 4 seq chunks of 512

_NC_CACHE = {}


def build_nc():
    import concourse.bass as bass
    import concourse.tile as tile
    from concourse import bacc, mybir

    f32 = mybir.dt.float32
    bf16 = mybir.dt.bfloat16
    EXP = mybir.ActivationFunctionType.Exp
    SCALE = float(D ** -0.5)

    nc = bacc.Bacc("TRN2", target_bir_lowering=False)

    xT = nc.dram_tensor("xT", [C, N], bf16, kind="ExternalInput")
    qkvwT = nc.dram_tensor("qkvwT", [C, 3 * FG], bf16, kind="ExternalInput")
    bqk = nc.dram_tensor("bqk", [6, 128, 1], f32, kind="ExternalInput")
    projwT = nc.dram_tensor("projwT", [FG, C], bf16, kind="ExternalInput")
    out = nc.dram_tensor("out", [N, C], f32, kind="ExternalOutput")

    from contextlib import ExitStack

    with tile.TileContext(nc) as tc, ExitStack() as ctx:
        const = ctx.enter_context(tc.tile_pool(name="const", bufs=1))
        mpsum = ctx.enter_context(tc.tile_pool(name="mpsum", bufs=2, space="PSUM"))
        spsum = ctx.enter_context(tc.tile_pool(name="spsum", bufs=2, space="PSUM"))
        opsum = ctx.enter_context(tc.tile_pool(name="opsum", bufs=4, space="PSUM"))
        attnp = ctx.enter_context(tc.tile_pool(name="attnp", bufs=12))
        smallp = ctx.enter_context(tc.tile_pool(name="smallp", bufs=4))
        outp = ctx.enter_context(tc.tile_pool(name="outp", bufs=3))

        x_sb = [const.tile([128, N], bf16, tag=f"x{i}", name=f"x{i}") for i in range(CC)]
        w_sb = [const.tile([128, 3 * FG], bf16, tag=f"w{i}", name=f"w{i}") for i in range(CC)]
        # Q^T rows in tiles 0-2, K^T rows in tiles 3-5
        kqT = [const.tile([128, N], bf16, tag=f"kq{j}", name=f"kq{j}") for j in range(6)]
        vp_sb = [const.tile([128, HPG, D + 1], bf16, tag=f"v{n}", name=f"v{n}") for n in range(NT)]
        pw_sb = [const.tile([128, C], bf16, tag=f"pw{j}", name=f"pw{j}") for j in range(3)]
        ao_sb = [const.tile([128, N], bf16, tag=f"ao{j}", name=f"ao{j}") for j in range(3)]
        bias_sb = [const.tile([128, 1], f32, tag=f"b{j}", name=f"b{j}") for j in range(6)]
        ones_sb = const.tile([1, 64], bf16, tag="ones", name="ones_sb")
        nc.vector.memset(ones_sb, 1.0)

        for i in range(CC):
            nc.sync.dma_start(out=x_sb[i], in_=xT[i * 128:(i + 1) * 128, :])
            nc.sync.dma_start(out=w_sb[i], in_=qkvwT[i * 128:(i + 1) * 128, :])
        for j in range(6):
            nc.sync.dma_start(out=bias_sb[j], in_=bqk[j])
        for j in range(3):
            nc.sync.dma_start(out=pw_sb[j], in_=projwT[j * 128:(j + 1) * 128, :])

        # V = x @ Wv in [seq, feat] layout, interleaved with a ones column per head
        for n in range(NT):
            ps = mpsum.tile([128, 512], f32, tag="bank")
            for cc in range(CC):
                nc.tensor.matmul(
                    ps[:, :FG],
                    lhsT=x_sb[cc][:, n * 128:(n + 1) * 128],
                    rhs=w_sb[cc][:, 2 * FG:3 * FG],
                    start=(cc == 0),
                    stop=(cc == CC - 1),
                )
            nc.vector.tensor_copy(
                out=vp_sb[n][:, :, 0:D],
                in_=ps[:, :FG].rearrange("p (h d) -> p h d", d=D),
            )
            nc.vector.memset(vp_sb[n][:, :, D:D + 1], 1.0)

        def qk_chunk(j):
            # Q^T/K^T feature rows j*128:(j+1)*128 over the whole sequence
            for q4 in range(NQ):
                ps = mpsum.tile([128, 512], f32, tag="bank")
                for cc in range(CC):
                    nc.tensor.matmul(
                        ps,
                        lhsT=w_sb[cc][:, j * 128:(j + 1) * 128],
                        rhs=x_sb[cc][:, q4 * 512:(q4 + 1) * 512],
                        start=(cc == 0),
                        stop=(cc == CC - 1),
                    )
                nc.vector.tensor_scalar_add(
                    out=kqT[j][:, q4 * 512:(q4 + 1) * 512],
                    in0=ps,
                    scalar1=bias_sb[j],
                )

        def attention(h):
            qt = kqT[h // 2]
            kt = kqT[3 + h // 2]
            off = (h % 2) * 64
            os_ = [opsum.tile([128, 512], f32, tag="obank", name="obank") for _ in range(NQ)]
            for kc in range(NT):
                k_stat = kt[off:off + 64, kc * 128:(kc + 1) * 128]
                v_stat = vp_sb[kc][:, h, :]
                for q4 in range(NQ):
                    sp = spsum.tile([128, 512], f32, tag="sbank")
                    nc.tensor.matmul(
                        sp,
                        lhsT=k_stat,
                        rhs=qt[off:off + 64, q4 * 512:(q4 + 1) * 512],
                        start=True,
                        stop=True,
                    )
                    at = attnp.tile([128, 512], bf16, tag="attnT")
                    nc.scalar.activation(out=at, in_=sp, func=EXP, scale=SCALE)
                    nc.tensor.matmul(
                        os_[q4][:D + 1, :],
                        lhsT=v_stat,
                        rhs=at,
                        start=(kc == 0),
                        stop=(kc == NT - 1),
                    )
            for q4 in range(NQ):
                r = smallp.tile([1, 512], f32, tag="recip")
                nc.vector.reciprocal(out=r, in_=os_[q4][D:D + 1, :])
                r_bf = smallp.tile([1, 512], bf16, tag="recipbf")
                nc.vector.tensor_copy(out=r_bf, in_=r)
                # broadcast recip across 64 partitions via PE outer product
                bp = spsum.tile([128, 512], f32, tag="sbank")
                nc.tensor.matmul(bp[:D, :], lhsT=ones_sb, rhs=r_bf, start=True, stop=True)
                rb = smallp.tile([64, 512], f32, tag="rb")
                nc.vector.tensor_copy(out=rb, in_=bp[:D, :])
                nc.vector.tensor_mul(
                    ao_sb[h // 2][off:off + 64, q4 * 512:(q4 + 1) * 512],
                    os_[q4][0:D, :],
                    rb,
                )

        # Emit Q/K chunks in head order so attention overlaps the tail of QKV
        for j, heads in ((0, (0, 1)), (1, (2, 3)), (2, (4, 5))):
            qk_chunk(j)
            qk_chunk(3 + j)
            for h in heads:
                attention(h)

        # out[n, :] = attn_out @ proj_w_slice.T  (partial; host adds biases)
        for n in range(NT):
            ob = outp.tile([128, C], f32, tag="osb")
            for f0, fw in ((0, 512), (512, 256)):
                pp = mpsum.tile([128, 512], f32, tag="bank")
                for j in range(3):
                    nc.tensor.matmul(
                        pp[:, :fw],
                        lhsT=ao_sb[j][:, n * 128:(n + 1) * 128],
                        rhs=pw_sb[j][:, f0:f0 + fw],
                        start=(j == 0),
                        stop=(j == 2),
                    )
                nc.vector.tensor_copy(out=ob[:, f0:f0 + fw], in_=pp[:, :fw])
            nc.sync.dma_start(out=out[n * 128:(n + 1) * 128, :], in_=ob)

    nc.compile()
    return nc


def _get_nc():
    if "nc" not in _NC_CACHE:
        _NC_CACHE["nc"] = build_nc()
    return _NC_CACHE["nc"]


def make_in_maps(x, qkv_w, qkv_b, proj_w):
    bf16 = ml_dtypes.bfloat16
    in_maps = []
    for core in range(8):
        b, g = divmod(core, 2)
        wq = qkv_w[0 * C + g * FG:0 * C + (g + 1) * FG]
        wk = qkv_w[1 * C + g * FG:1 * C + (g + 1) * FG]
        wv = qkv_w[2 * C + g * FG:2 * C + (g + 1) * FG]
        wqkvT = np.ascontiguousarray(np.concatenate([wq, wk, wv], 0).T).astype(bf16)
        xT = np.ascontiguousarray(x[b].T).astype(bf16)
        bq = qkv_b[0 * C + g * FG:0 * C + (g + 1) * FG]
        bk = qkv_b[1 * C + g * FG:1 * C + (g + 1) * FG]
        bqk = np.concatenate([bq, bk]).astype(np.float32).reshape(6, 128, 1)
        wpT = np.ascontiguousarray(proj_w[:, g * FG:(g + 1) * FG].T).astype(bf16)
        in_maps.append({"xT": xT, "qkvwT": wqkvT, "bqk": bqk, "projwT": wpT})
    return in_maps


def combine(parts, qkv_b, proj_w, proj_b):
    corr = (qkv_b[2 * C:] @ proj_w.T + proj_b).astype(np.float32)
    out = np.empty((B, N, C), np.float32)
    for b in range(B):
        out[b] = parts[2 * b] + parts[2 * b + 1] + corr
    return out


def kernel(**inputs):
    x = np.asarray(inputs["x"], np.float32)
    qkv_w = np.asarray(inputs["qkv_w"], np.float32)
    qkv_b = np.asarray(inputs["qkv_b"], np.float32)
    proj_w = np.asarray(inputs["proj_w"], np.float32)
    proj_b = np.asarray(inputs["proj_b"], np.float32)

    from concourse.bass_utils import run_bass_kernel_spmd

    nc = _get_nc()
    in_maps = make_in_maps(x, qkv_w, qkv_b, proj_w)
    res = run_bass_kernel_spmd(nc, in_maps, core_ids=list(range(8)))
    parts = [r["out"] for r in res.results]
    return combine(parts, qkv_b, proj_w, proj_b)
